# revision 12
# baseline (speedup 1.0000x reference)
"""GroupSort(2) Trainium2 Bass kernel.

The reference module
    diff = relu(w_diff @ x);  out = x + w_expand @ diff
with the fixed pair-difference weights is algebraically a pairwise sort:
    out[2k]   = min(x[2k], x[2k+1])
    out[2k+1] = max(x[2k], x[2k+1])
so the kernel is pure elementwise min/max — no matmuls.

Sharding: pure data parallel, batch 32 -> 8 cores x 4.
Per-core layout: x shard [4, 256, 64, 64] viewed as [4, 128, 2, 4096]
(channel pair k lives on partition k; even/odd members side by side).

The kernel is chip-HBM-bound: 8 cores x (16 MiB in + 16 MiB out) =
256 MiB at ~2.9 TB/s ~= 90 us.  The raw-bass pipeline below keeps the
single SP DGE queue saturated (loads run ahead of DVE-gated stores via
descriptor-attached sem waits) and minimizes preamble/epilogue time.
"""

import numpy as np

import bass_rust
import concourse.mybir as mybir
from concourse.bass import Bass
from concourse.tile import TileContext
from concourse.bass_utils import run_bass_kernel_spmd

N_CORES = 8
B, C, H, W = 32, 256, 64, 64
BS = B // N_CORES          # batches per core
P = 128                    # channel pairs -> SBUF partitions
HW = H * W                 # 4096
K = 2048                   # hw chunk per tile
NCHUNK = BS * (HW // K)    # 8
B_IN = 3                   # tin slots
B_OUT = 4                  # tout slots
LAG = 2                    # stores trail loads by LAG chunks in the queue
DT = mybir.dt.float32

_nc_cache = {}


def _build_raw():
    nc = Bass()
    x = nc.declare_dram_parameter("x", [BS, P, 2, HW], DT, isOutput=False)
    out = nc.declare_dram_parameter("out", [BS, P, 2, HW], DT, isOutput=True)

    def src(i):
        b, jj = divmod(i, HW // K)
        return x[b, :, :, jj * K : (jj + 1) * K]

    def dst(i):
        b, jj = divmod(i, HW // K)
        return out[b, :, :, jj * K : (jj + 1) * K]

    import contextlib

    # First/last chunk compute+store are split column-wise so sub-stores
    # release as soon as their slice of DVE work lands: the first store
    # stops stalling the DGE queue behind chunk 0's full min+max pair,
    # and the drain stops serializing behind the last chunk's full pair.
    SPLIT = {0: 2, NCHUNK - 1: 4}
    units = []                      # (chunk, col_off, col_len) in DVE order
    for c in range(NCHUNK):
        nsub = SPLIT.get(c, 1)
        w = K // nsub
        for u in range(nsub):
            units.append((c, u * w, w))
    dv_after = {}                   # chunk -> dv value once fully computed
    chunk_units = {c: [] for c in range(NCHUNK)}
    for idx, (c, o, w) in enumerate(units):
        dv_after[c] = idx + 1
        chunk_units[c].append((idx, o, w))

    with contextlib.ExitStack() as stack:
        block = stack.enter_context(nc.Block())
        # Per-chunk completion sems: DMA slice completions from the 16
        # HWDGE engines interleave across transfers, so one shared counter
        # cannot order chunk boundaries (race detector rightly objects).
        ld = [stack.enter_context(nc.semaphore(f"ld{i}")) for i in range(NCHUNK)]
        st = [stack.enter_context(nc.semaphore(f"st{i}")) for i in range(NCHUNK)]
        dv_sem = stack.enter_context(nc.semaphore("dv_sem"))
        tin = stack.enter_context(nc.sbuf_tensor("tin", [P, B_IN, 2, K], DT))
        tout = stack.enter_context(nc.sbuf_tensor("tout", [P, B_OUT, 2, K], DT))

        @block.sync
        def _(sync):
            def store(si):
                b, jj = divmod(si, HW // K)
                for idx, o, w in chunk_units[si]:
                    ins = sync.dma_start(
                        out=out[b, :, :, jj * K + o : jj * K + o + w],
                        in_=tout[:, si % B_OUT, :, o : o + w],
                    )
                    ins._wait_ge(dv_sem, idx + 1)
                    ins.then_inc(st[si], 16)

            for i in range(NCHUNK):
                if i - LAG >= 0:
                    store(i - LAG)
                ins = sync.dma_start(out=tin[:, i % B_IN], in_=src(i))
                if i >= B_IN:
                    # slot reuse: DVE must have consumed chunk i-B_IN
                    ins._wait_ge(dv_sem, dv_after[i - B_IN])
                ins.then_inc(ld[i], 16)
            for si in range(NCHUNK - LAG, NCHUNK):
                store(si)
            for si in range(NCHUNK):
                sync.wait_ge(st[si], 16 * len(chunk_units[si]))

        @block.vector
        def _(vector):
            for i in range(NCHUNK):
                if i >= B_OUT:
                    # tout slot reuse: stores of chunk i-B_OUT finished
                    vector.wait_ge(st[i - B_OUT], 16 * len(chunk_units[i - B_OUT]))
                for n, (idx, o, w) in enumerate(chunk_units[i]):
                    ins = vector.tensor_tensor(
                        out=tout[:, i % B_OUT, 0, o : o + w],
                        in0=tin[:, i % B_IN, 0, o : o + w],
                        in1=tin[:, i % B_IN, 1, o : o + w],
                        op=mybir.AluOpType.min,
                    )
                    if n == 0:
                        ins._wait_ge(ld[i], 16)
                    vector.tensor_tensor(
                        out=tout[:, i % B_OUT, 1, o : o + w],
                        in0=tin[:, i % B_IN, 0, o : o + w],
                        in1=tin[:, i % B_IN, 1, o : o + w],
                        op=mybir.AluOpType.max,
                    ).then_inc(dv_sem, 1)

    bass_rust.generate_event_semaphores(nc)
    nc.finalize()
    return nc


def _build_tile():
    nc = Bass()
    x = nc.declare_dram_parameter("x", [BS, P, 2, HW], DT, isOutput=False)
    out = nc.declare_dram_parameter("out", [BS, P, 2, HW], DT, isOutput=True)
    with TileContext(nc) as tc:
        with (
            tc.tile_pool(name="pin", bufs=4) as pin,
            tc.tile_pool(name="pout", bufs=3) as pout,
        ):
            for b in range(BS):
                for j in range(0, HW, K):
                    tin = pin.tile([P, 2, K], DT)
                    tout = pout.tile([P, 2, K], DT)
                    nc.sync.dma_start(out=tin, in_=x[b, :, :, j : j + K])
                    nc.vector.tensor_tensor(
                        out=tout[:, 0, :], in0=tin[:, 0, :], in1=tin[:, 1, :],
                        op=mybir.AluOpType.min,
                    )
                    nc.vector.tensor_tensor(
                        out=tout[:, 1, :], in0=tin[:, 0, :], in1=tin[:, 1, :],
                        op=mybir.AluOpType.max,
                    )
                    nc.sync.dma_start(out=out[b, :, :, j : j + K], in_=tout)
    # TRN2 allows at most one sync-wait per instruction; Tile can attach
    # several (load sem + slot-release sem). Split the excess onto
    # InstEventSemaphores or neuronxcc codegen rejects the TensorTensors.
    bass_rust.generate_event_semaphores(nc)
    nc.finalize()
    return nc


def _build(variant="raw"):
    if variant not in _nc_cache:
        _nc_cache[variant] = _build_raw() if variant == "raw" else _build_tile()
    return _nc_cache[variant]


def _run(x, trace=False, variant="raw", **kwargs):
    nc = _build(variant)
    xs = np.ascontiguousarray(np.asarray(x, dtype=np.float32)).reshape(
        N_CORES, BS, P, 2, HW
    )
    in_maps = [{"x": xs[i]} for i in range(N_CORES)]
    res = run_bass_kernel_spmd(
        nc, in_maps, core_ids=list(range(N_CORES)), trace=trace, **kwargs
    )
    out = np.stack([r["out"] for r in res.results], axis=0).reshape(B, C, H, W)
    return out, res


def kernel(x, **_unused_weights):
    out, _ = _run(x)
    return out


# revision 13
# speedup vs baseline: 1.1133x; 1.1133x over previous
"""GroupSort(2) Trainium2 Bass kernel.

The reference module
    diff = relu(w_diff @ x);  out = x + w_expand @ diff
with the fixed pair-difference weights is algebraically a pairwise sort:
    out[2k]   = min(x[2k], x[2k+1])
    out[2k+1] = max(x[2k], x[2k+1])
so the kernel is pure elementwise min/max — no matmuls.

Sharding: pure data parallel, batch 32 -> 8 cores x 4.
Per-core layout: x shard [4, 256, 64, 64] viewed as [4, 128, 2, 4096]
(channel pair k lives on partition k; even/odd members side by side).

The kernel is chip-HBM-bound: 8 cores x (16 MiB in + 16 MiB out) =
256 MiB at ~2.9 TB/s ~= 90 us.  The raw-bass pipeline below keeps the
single SP DGE queue saturated (loads run ahead of DVE-gated stores via
descriptor-attached sem waits) and minimizes preamble/epilogue time.
"""

import numpy as np

import bass_rust
import concourse.mybir as mybir
from concourse.bass import Bass
from concourse.tile import TileContext
from concourse.bass_utils import run_bass_kernel_spmd

N_CORES = 8
B, C, H, W = 32, 256, 64, 64
BS = B // N_CORES          # batches per core
P = 128                    # channel pairs -> SBUF partitions
HW = H * W                 # 4096
K = 2048                   # hw chunk per tile
NCHUNK = BS * (HW // K)    # 8
B_IN = 3                   # tin slots
B_OUT = 4                  # tout slots
LAG = 2                    # stores trail loads by LAG chunks in the queue
DT = mybir.dt.float32

_nc_cache = {}


def _build_raw():
    nc = Bass()
    x = nc.declare_dram_parameter("x", [BS, P, 2, HW], DT, isOutput=False)
    out = nc.declare_dram_parameter("out", [BS, P, 2, HW], DT, isOutput=True)

    def src(i):
        b, jj = divmod(i, HW // K)
        return x[b, :, :, jj * K : (jj + 1) * K]

    def dst(i):
        b, jj = divmod(i, HW // K)
        return out[b, :, :, jj * K : (jj + 1) * K]

    import contextlib

    # First/last chunk compute+store are split column-wise so sub-stores
    # release as soon as their slice of DVE work lands: the first store
    # stops stalling the DGE queue behind chunk 0's full min+max pair,
    # and the drain stops serializing behind the last chunk's full pair.
    # NOTE: chunk 0 must stay unsplit — the first store's ~1.4us stall at
    # the DGE queue head seeds a direction-batched phase across the 8
    # cores that sustains ~2.98 TB/s chip HBM; making S0 arrive "in time"
    # (splitting chunk 0) drops the whole stream to ~2.6 TB/s (measured).
    SPLIT = {NCHUNK - 1: 4}
    units = []                      # (chunk, col_off, col_len) in DVE order
    for c in range(NCHUNK):
        nsub = SPLIT.get(c, 1)
        w = K // nsub
        for u in range(nsub):
            units.append((c, u * w, w))
    dv_after = {}                   # chunk -> dv value once fully computed
    chunk_units = {c: [] for c in range(NCHUNK)}
    for idx, (c, o, w) in enumerate(units):
        dv_after[c] = idx + 1
        chunk_units[c].append((idx, o, w))

    with contextlib.ExitStack() as stack:
        block = stack.enter_context(nc.Block())
        # Per-chunk completion sems: DMA slice completions from the 16
        # HWDGE engines interleave across transfers, so one shared counter
        # cannot order chunk boundaries (race detector rightly objects).
        ld = [stack.enter_context(nc.semaphore(f"ld{i}")) for i in range(NCHUNK)]
        st = [stack.enter_context(nc.semaphore(f"st{i}")) for i in range(NCHUNK)]
        dv_sem = stack.enter_context(nc.semaphore("dv_sem"))
        tin = stack.enter_context(nc.sbuf_tensor("tin", [P, B_IN, 2, K], DT))
        tout = stack.enter_context(nc.sbuf_tensor("tout", [P, B_OUT, 2, K], DT))

        @block.sync
        def _(sync):
            def store(si):
                b, jj = divmod(si, HW // K)
                for idx, o, w in chunk_units[si]:
                    ins = sync.dma_start(
                        out=out[b, :, :, jj * K + o : jj * K + o + w],
                        in_=tout[:, si % B_OUT, :, o : o + w],
                    )
                    ins._wait_ge(dv_sem, idx + 1)
                    ins.then_inc(st[si], 16)

            for i in range(NCHUNK):
                if i - LAG >= 0:
                    store(i - LAG)
                ins = sync.dma_start(out=tin[:, i % B_IN], in_=src(i))
                if i >= B_IN:
                    # slot reuse: DVE must have consumed chunk i-B_IN
                    ins._wait_ge(dv_sem, dv_after[i - B_IN])
                ins.then_inc(ld[i], 16)
            for si in range(NCHUNK - LAG, NCHUNK):
                store(si)
            for si in range(NCHUNK):
                sync.wait_ge(st[si], 16 * len(chunk_units[si]))

        @block.vector
        def _(vector):
            for i in range(NCHUNK):
                if i >= B_OUT:
                    # tout slot reuse: stores of chunk i-B_OUT finished
                    vector.wait_ge(st[i - B_OUT], 16 * len(chunk_units[i - B_OUT]))
                for n, (idx, o, w) in enumerate(chunk_units[i]):
                    ins = vector.tensor_tensor(
                        out=tout[:, i % B_OUT, 0, o : o + w],
                        in0=tin[:, i % B_IN, 0, o : o + w],
                        in1=tin[:, i % B_IN, 1, o : o + w],
                        op=mybir.AluOpType.min,
                    )
                    if n == 0:
                        ins._wait_ge(ld[i], 16)
                    vector.tensor_tensor(
                        out=tout[:, i % B_OUT, 1, o : o + w],
                        in0=tin[:, i % B_IN, 0, o : o + w],
                        in1=tin[:, i % B_IN, 1, o : o + w],
                        op=mybir.AluOpType.max,
                    ).then_inc(dv_sem, 1)

    bass_rust.generate_event_semaphores(nc)
    nc.finalize()
    return nc


def _build_tile():
    nc = Bass()
    x = nc.declare_dram_parameter("x", [BS, P, 2, HW], DT, isOutput=False)
    out = nc.declare_dram_parameter("out", [BS, P, 2, HW], DT, isOutput=True)
    with TileContext(nc) as tc:
        with (
            tc.tile_pool(name="pin", bufs=4) as pin,
            tc.tile_pool(name="pout", bufs=3) as pout,
        ):
            for b in range(BS):
                for j in range(0, HW, K):
                    tin = pin.tile([P, 2, K], DT)
                    tout = pout.tile([P, 2, K], DT)
                    nc.sync.dma_start(out=tin, in_=x[b, :, :, j : j + K])
                    nc.vector.tensor_tensor(
                        out=tout[:, 0, :], in0=tin[:, 0, :], in1=tin[:, 1, :],
                        op=mybir.AluOpType.min,
                    )
                    nc.vector.tensor_tensor(
                        out=tout[:, 1, :], in0=tin[:, 0, :], in1=tin[:, 1, :],
                        op=mybir.AluOpType.max,
                    )
                    nc.sync.dma_start(out=out[b, :, :, j : j + K], in_=tout)
    # TRN2 allows at most one sync-wait per instruction; Tile can attach
    # several (load sem + slot-release sem). Split the excess onto
    # InstEventSemaphores or neuronxcc codegen rejects the TensorTensors.
    bass_rust.generate_event_semaphores(nc)
    nc.finalize()
    return nc


def _build(variant="raw"):
    if variant not in _nc_cache:
        _nc_cache[variant] = _build_raw() if variant == "raw" else _build_tile()
    return _nc_cache[variant]


def _run(x, trace=False, variant="raw", **kwargs):
    nc = _build(variant)
    xs = np.ascontiguousarray(np.asarray(x, dtype=np.float32)).reshape(
        N_CORES, BS, P, 2, HW
    )
    in_maps = [{"x": xs[i]} for i in range(N_CORES)]
    res = run_bass_kernel_spmd(
        nc, in_maps, core_ids=list(range(N_CORES)), trace=trace, **kwargs
    )
    out = np.stack([r["out"] for r in res.results], axis=0).reshape(B, C, H, W)
    return out, res


def kernel(x, **_unused_weights):
    out, _ = _run(x)
    return out
